# revision 42
# baseline (speedup 1.0000x reference)
"""Trainium2 Bass kernel for nn_DGAD_net (vq_codebook).

Strategy (v3)
-------------
The reference's dominant cost is ``einsum('bchw,oc->bohw', inter, w).mean((2,3))``
followed by tiny MLPs.  The einsum and the spatial mean commute (both linear),
so on device we only *sum-pool* the feature maps and fold ``w / (H*W)`` into the
first matmul.  v2 hit the f32 DMA roofline (120 MB/core at ~368 GB/s = 326 us),
so v3 halves the bytes:

- Features are host-cast to bf16 and host-packed so that every DMA is one
  contiguous >=12 KiB run per partition (descriptor-efficient; channel
  c = g*128 + p to match the weight packing).
- ``sfc_w1 @ (shallow_conv_w / HW)`` is folded on the host: kills the 0.25 MB
  conv weight + the whole 512-wide "sh" matmul stage.
- Pooling: InstTensorReduce has no DVE fast modes (1x = 123 G elem/s, would
  bottleneck at ~235 us), so the 16x49 spatial factor is reduced with a
  tensor_tensor add tree (2x_1p mode, bf16 packed operands) down to 1x49,
  then one 1x reduce.  ~6.7 Kcycles per 8-sample tile.

Data parallel over 8 NeuronCores: batch 512 -> 64 per core.  Each core
returns a [1, 264] row of per-sample partials:
  [osvdd_main(64) | csvdd_main(64) | osvdd_aug(64) | csvdd_aug(64)
   | sum(q0*ls1)(4) | sum(q1*ls0)(4)]
and the host reduces those to the [4, 1] output.

The distill term only needs softmax/log_softmax of ``sim`` which are invariant
to per-row shifts, so the device computes ``score[b,k] = 2 t.p_k - ||p_k||^2``
(skipping ``-||t||^2``) -- same softmax, same argmax.

Streams xi -> ai -> xo -> ao alternate over both HWDGE rings (weights up
front, split across rings); the long shallow/texture/distill chains overlap
the later streams so the kernel tail is only the short aug-origin chain.

This walrus build only encodes ONE sync wait per instruction; ``_split_waits``
rewrites the traced BIR, moving excess waits onto preceding same-engine NOPs.
"""

import sys

for _p in ("/opt/trn_rl_repo", "/root/.axon_site/_ro/trn_rl_repo"):
    if _p not in sys.path:
        sys.path.append(_p)

import numpy as np

B, CI, HW2, CO, HO2, D, DOM = 512, 256, 784, 512, 49, 64, 4
NCORE = 8
BC = B // NCORE  # 64 samples per core
TS = 8           # inter samples per tile
NT = BC // TS    # 8 inter tiles per stream
EPOCHS = 30
W_TEMP, T_TEMP = 0.7, 0.4
_SCHED = np.concatenate(
    [np.linspace(W_TEMP, T_TEMP, int(EPOCHS * 0.25)),
     np.ones(EPOCHS - int(EPOCHS * 0.25)) * T_TEMP]
)

_NC = None  # built once per process
_ctr = [0]


def _split_waits(nc, mybir, cap=1):
    """Move excess sync waits onto preceding same-engine NOPs (this
    walrus encodes at most `cap` waits per instruction).  Same-engine
    program order preserves semantics exactly."""
    for f in nc.m.functions:
        for bb in f.blocks:
            new = []
            for inst in bb.instructions:
                si = inst.sync_info
                if si is not None and si.on_wait and len(si.on_wait) > cap:
                    waits = list(si.on_wait)
                    excess, keep = waits[:-cap], waits[-cap:]
                    while excess:
                        chunk, excess = excess[:cap], excess[cap:]
                        nop = mybir.InstNoOp(
                            name=f"I-wsplit-{_ctr[0]}", ins=[], outs=[]
                        )
                        _ctr[0] += 1
                        nop.engine = inst.engine
                        nop.sync_info = mybir.SyncInfo(on_wait=chunk, on_update=[])
                        new.append(nop)
                    inst.sync_info = mybir.SyncInfo(
                        on_wait=keep, on_update=list(si.on_update)
                    )
                new.append(inst)
            bb.instructions = new


def _build_nc():
    import concourse.bass as bass
    import concourse.tile as tile
    from concourse import mybir
    from contextlib import ExitStack

    AF = mybir.ActivationFunctionType
    AL = mybir.AluOpType
    AX = mybir.AxisListType
    f32 = mybir.dt.float32
    bf16 = mybir.dt.bfloat16

    nc = bass.Bass(trn_type="TRN2")

    # ---- DRAM I/O ----
    # inter: [p, tile, b(8), g(2), s(784)] bf16, channel c = g*128 + p
    xi = nc.dram_tensor("xi", [128, NT, TS * 2 * HW2], bf16, kind="ExternalInput")
    ai = nc.dram_tensor("ai", [128, NT, TS * 2 * HW2], bf16, kind="ExternalInput")
    # origin: [p, g(4), b(64), s(49)] bf16, channel c = g*128 + p
    xo = nc.dram_tensor("xo", [128, 4 * BC * HO2], bf16, kind="ExternalInput")
    ao = nc.dram_tensor("ao", [128, 4 * BC * HO2], bf16, kind="ExternalInput")
    # weights: host-packed flat [128, X] (per-partition contiguous)
    w_m = nc.dram_tensor("w_m", [128, 2 * 1024], bf16, kind="ExternalInput")
    w_s2 = nc.dram_tensor("w_s2", [128, 8 * 512], bf16, kind="ExternalInput")
    w_s3 = nc.dram_tensor("w_s3", [128, 4 * 64], bf16, kind="ExternalInput")
    w_o1 = nc.dram_tensor("w_o1", [128, 4 * 1024], bf16, kind="ExternalInput")
    w_o2 = nc.dram_tensor("w_o2", [128, 8 * 512], bf16, kind="ExternalInput")
    w_o3 = nc.dram_tensor("w_o3", [128, 4 * 64], bf16, kind="ExternalInput")
    # small weights compacted: [64, 384] bf16 = t1|t2|c1|c2|q1|q2,
    # [64, 67] f32 = idm | b_t1 | b_cn | invt
    w_sm = nc.dram_tensor("w_sm", [64, 384], bf16, kind="ExternalInput")
    w_sf = nc.dram_tensor("w_sf", [64, 67], f32, kind="ExternalInput")
    w_pc = nc.dram_tensor("w_pc", [4, 64], bf16, kind="ExternalInput")
    w_p2 = nc.dram_tensor("w_p2", [65, 4], f32, kind="ExternalInput")
    out = nc.dram_tensor("out", [1, 264], f32, kind="ExternalOutput")

    from concourse.tile import add_dep_helper

    with tile.TileContext(nc) as tc:
        with ExitStack() as ctx:
            wp = ctx.enter_context(tc.tile_pool(name="wp", bufs=1))
            iop = ctx.enter_context(tc.tile_pool(name="iop", bufs=4))
            orp = ctx.enter_context(tc.tile_pool(name="orp", bufs=2))
            plp = ctx.enter_context(tc.tile_pool(name="plp", bufs=1))
            ap = ctx.enter_context(tc.tile_pool(name="ap", bufs=1))
            sp = ctx.enter_context(tc.tile_pool(name="sp", bufs=1))
            psA = ctx.enter_context(tc.tile_pool(name="psA", bufs=4, space="PSUM"))
            psB = ctx.enter_context(tc.tile_pool(name="psB", bufs=1, space="PSUM"))

            # ---- weights: on the ACT (scalar) HWDGE ring, which is free of
            # other DMA traffic; they stream concurrently with the first
            # feature tiles on the sync ring and are done long before any
            # ACT compute shows up on that sequencer ----
            def wtile(h, shape, tag, dt=f32):
                t = wp.tile(shape, dt, tag=tag)
                dst = t[:]
                if len(shape) == 3:
                    dst = dst.rearrange("p m j -> p (m j)")
                nc.scalar.dma_start(dst, h[:])
                return t

            W = {}

            def load_weights():
                W["m"] = wtile(w_m, [128, 2, 1024], "w_m", bf16)
                W["o1"] = wtile(w_o1, [128, 4, 1024], "w_o1", bf16)
                W["s2"] = wtile(w_s2, [128, 8, 512], "w_s2", bf16)
                W["o2"] = wtile(w_o2, [128, 8, 512], "w_o2", bf16)
                W["s3"] = wtile(w_s3, [128, 4, 64], "w_s3", bf16)
                W["o3"] = wtile(w_o3, [128, 4, 64], "w_o3", bf16)
                W["sm"] = wtile(w_sm, [64, 384], "w_sm", bf16)
                W["sf"] = wtile(w_sf, [64, 67], "w_sf")
                W["pc"] = wtile(w_pc, [4, 64], "w_pc", bf16)
                W["p2"] = wtile(w_p2, [65, 4], "w_p2")

            ones_sb = wp.tile([64, 1], f32, tag="ones")
            nc.vector.memset(ones_sb[:], 1.0)
            out_sb = wp.tile([1, 264], f32, tag="out_sb")

            # ---- pooling.  The early stream portion (xi/xo, done before any
            # ACT compute exists) alternates over both HWDGE rings to hide
            # per-DMA completion tails; the late portion (ai/ao) stays on the
            # sync ring, whose sequencer runs no compute.  The add tree runs
            # in place inside the io tile so each tile's tree is fully
            # independent. ----
            def pool_inter(xh, tag, ring2=False, after_first_pair=None):
                """[128, 2, BC] bf16 pooled sums; channel c = g*128 + p."""
                dst = plp.tile([128, 2, BC], bf16, tag=tag)
                for t in range(NT):
                    tl = iop.tile([128, 2 * TS, 16, HO2], bf16, tag="io")
                    eng = nc.scalar if (ring2 and t % 2 == 1) else nc.sync
                    eng.dma_start(
                        tl[:].rearrange("p m k s -> p (m k s)"), xh[:, t, :]
                    )
                    if t == 1 and after_first_pair is not None:
                        after_first_pair()
                    # in-place add tree over the 16 spatial chunks (2x_1p bf16)
                    nc.vector.tensor_add(
                        tl[:, :, 0:8, :], tl[:, :, 0:8, :], tl[:, :, 8:16, :]
                    )
                    nc.vector.tensor_add(
                        tl[:, :, 0:4, :], tl[:, :, 0:4, :], tl[:, :, 4:8, :]
                    )
                    nc.vector.tensor_add(
                        tl[:, :, 0:2, :], tl[:, :, 0:2, :], tl[:, :, 2:4, :]
                    )
                    nc.vector.tensor_add(
                        tl[:, :, 0:1, :], tl[:, :, 0:1, :], tl[:, :, 1:2, :]
                    )
                    with nc.allow_low_precision(reason="pooled sums in bf16"):
                        nc.vector.reduce_sum(
                            dst[:, :, t * TS:(t + 1) * TS]
                            .rearrange("p g b -> p b g"),
                            tl[:, :, 0:1, :],
                            axis=AX.X,
                        )
                return dst

            def pool_origin(xh, tag, ring2=False):
                """[128, 4, BC] bf16 pooled sums; channel c = g*128 + p."""
                dst = plp.tile([128, 4, BC], bf16, tag=tag)
                for g in range(4):  # one channel-group per chunk
                    tl = orp.tile([128, BC, HO2], bf16, tag="or")
                    eng = nc.scalar if (ring2 and g % 2 == 1) else nc.sync
                    eng.dma_start(
                        tl[:].rearrange("p b s -> p (b s)"),
                        xh[:, g * BC * HO2:(g + 1) * BC * HO2],
                    )
                    nc.vector.tensor_add(
                        tl[:, :, 0:24], tl[:, :, 0:24], tl[:, :, 24:48]
                    )
                    nc.vector.tensor_add(
                        tl[:, :, 0:12], tl[:, :, 0:12], tl[:, :, 12:24]
                    )
                    with nc.allow_low_precision(reason="pooled sums in bf16"):
                        nc.vector.reduce_sum(
                            dst[:, g, :], tl[:, :, 0:12], axis=AX.X,
                        )
                    nc.vector.tensor_add(
                        dst[:, g, :], dst[:, g, :], tl[:, :, 48]
                    )
                return dst

            # ---- MLP helpers ----
            def chunk_layer(w_sb, ins, nout_chunks, outw, r, nm, act=True):
                """outT chunks [outw, BC] = Lrelu( sum_m w_sb[:, m, chunk] @ ins[m] )."""
                outs = []
                nin = len(ins)
                for m2 in range(nout_chunks):
                    ps = psA.tile([128, BC], f32, tag="mm")
                    for m in range(nin):
                        nc.tensor.matmul(
                            ps[:outw, :],
                            w_sb[:, m, m2 * outw:(m2 + 1) * outw],
                            ins[m],
                            start=(m == 0),
                            stop=(m == nin - 1),
                        )
                    tl = ap.tile([outw, BC], bf16, tag=f"r{r}{nm}{m2}")
                    if act:
                        nc.scalar.activation(tl[:], ps[:outw, :], AF.Lrelu,
                                             alpha=0.01)
                    else:
                        nc.scalar.copy(tl[:], ps[:outw, :])
                    outs.append(tl[:])
                return outs

            def small_mm(lhsT, rhs, r, nm, act=True, bias=None, extra=None):
                """[64, BC] = act(lhsT.T @ rhs [+ extra matmul] + bias)."""
                ps = psA.tile([128, BC], f32, tag="mm")
                nc.tensor.matmul(
                    ps[:64, :], lhsT[:], rhs[:],
                    start=True, stop=(extra is None),
                )
                if extra is not None:
                    nc.tensor.matmul(
                        ps[:64, :], extra[0][:], extra[1][:], start=False, stop=True
                    )
                tl = ap.tile([64, BC], bf16, tag=f"r{r}{nm}")
                if bias is not None:
                    nc.scalar.activation(
                        tl[:], ps[:64, :], AF.Lrelu, bias=bias[:], alpha=0.01
                    )
                else:
                    nc.scalar.activation(tl[:], ps[:64, :], AF.Lrelu, alpha=0.01)
                return tl

            def svdd_row(featT, r, nm, off):
                """out_sb[0, off:off+64] = || featT[:, b] - center ||^2 per b.

                Round 0 runs mid-stream: its ops go to gpsimd (slow but off
                the vector queue, which must stay clear for the add trees).
                Round 1 runs in the tail where vector is idle and fast."""
                ve = nc.gpsimd if r == 0 else nc.vector
                df = sp.tile([64, BC], f32, tag=f"r{r}{nm}df")
                ve.tensor_scalar(df[:], featT[:], bcn_sb[:], None, op0=AL.add)
                sq = sp.tile([64, BC], f32, tag=f"r{r}{nm}sq")
                ve.tensor_mul(sq[:], df[:], df[:])
                pr = psB.tile([1, BC], f32, tag="row")
                nc.tensor.matmul(pr[:], ones_sb[:], sq[:])
                # PSUM source: gpsimd cannot read PSUM -> ACT for round 0
                if r == 0:
                    nc.scalar.copy(out_sb[:, off:off + BC], pr[:])
                else:
                    nc.vector.tensor_copy(out_sb[:, off:off + BC], pr[:])

            def stats_exp(score, mx, r):
                """Exp phase for teacher q and student log-softmax (both
                rounds' Exps are emitted together: one ACT table load)."""
                nb = sp.tile([64, 1], f32, tag=f"nb{r}")
                nc.vector.tensor_scalar(nb[:], mx[:], it_sb[:], -1.0,
                                        op0=AL.mult, op1=AL.mult)
                e = sp.tile([64, 4], f32, tag=f"te{r}")
                es = sp.tile([64, 1], f32, tag=f"tes{r}")
                nc.scalar.activation(e[:], score[:], AF.Exp, bias=nb[:],
                                     scale=it_sb[:], accum_out=es[:])
                snb = sp.tile([64, 1], f32, tag=f"snb{r}")
                nc.vector.tensor_scalar(snb[:], mx[:], -1.0, None, op0=AL.mult)
                se = sp.tile([64, 4], f32, tag=f"se{r}")
                ses = sp.tile([64, 1], f32, tag=f"ses{r}")
                nc.scalar.activation(se[:], score[:], AF.Exp, bias=snb[:],
                                     accum_out=ses[:])
                return e, es, ses

            def stats_fin(score, mx, e, es, ses, r):
                """Ln phase + q/ls products (vector)."""
                rc = sp.tile([64, 1], f32, tag=f"trc{r}")
                nc.vector.reciprocal(rc[:], es[:])
                q = sp.tile([64, 4], f32, tag=f"tq{r}")
                nc.vector.tensor_scalar(q[:], e[:], rc[:], None, op0=AL.mult)
                ln = sp.tile([64, 1], f32, tag=f"sln{r}")
                nc.scalar.activation(ln[:], ses[:], AF.Ln)
                lse = sp.tile([64, 1], f32, tag=f"slse{r}")
                nc.vector.tensor_add(lse[:], ln[:], mx[:])
                ls = sp.tile([64, 4], f32, tag=f"sls{r}")
                nc.vector.tensor_scalar(ls[:], score[:], lse[:], None,
                                        op0=AL.subtract)
                return q, ls

            def shallow_to_score(pi, r):
                """pooled inter -> score/mx for round r (stats done later)."""
                a1 = chunk_layer(m_sb, [pi[:, g, :] for g in range(2)], 8, 128,
                                 r, "a1")
                a2 = chunk_layer(s2_sb, a1, 4, 128, r, "a2")
                ps = psA.tile([128, BC], f32, tag="mm")
                for m in range(4):
                    nc.tensor.matmul(ps[:64, :], s3_sb[:, m, :], a2[m],
                                     start=(m == 0), stop=(m == 3))
                sT = ap.tile([64, BC], bf16, tag=f"r{r}sT")
                nc.scalar.activation(sT[:], ps[:64, :], AF.Lrelu, alpha=0.01)

                # texture MLP (concat folded into w_t1 + bias)
                t1 = small_mm(t1_sb, sT, r, "t1", bias=bt1_sb)
                ps = psA.tile([128, BC], f32, tag="mm")
                nc.tensor.matmul(ps[:64, :], t2_sb[:], t1[:])
                tx = ap.tile([65, BC], f32, tag=f"r{r}tx")
                nc.scalar.activation(tx[0:64, :], ps[:64, :], AF.Lrelu, alpha=0.01)
                nc.vector.memset(tx[64:65, :], 1.0)

                # score[b, k] = 2 t.p_k - ||p_k||^2.  Everything below runs
                # on gpsimd: these ops wait on the PE chain, and on the
                # in-order vector queue they would block later tree work.
                pss = psB.tile([64, 4], f32, tag="sc")
                nc.tensor.matmul(pss[:], tx[:], p2_sb[:])
                score = sp.tile([64, 4], f32, tag=f"score{r}")
                nc.scalar.copy(score[:], pss[:])
                mx = sp.tile([64, 1], f32, tag=f"mx{r}")
                nc.vector.reduce_max(mx[:], score[:], axis=AX.X)
                return score, mx

            def onehot_T(score, mx, r):
                """argmax one-hot, transposed -- gpsimd + PE only."""
                oh1 = sp.tile([64, 4], f32, tag=f"oh{r}")
                nc.gpsimd.tensor_scalar(oh1[:], score[:], mx[:], None,
                                        op0=AL.is_ge)
                psT = psB.tile([4, 64], f32, tag="ohT")
                nc.tensor.transpose(psT[:], oh1[:], id_sb[:])
                ohT = sp.tile([4, 64], bf16, tag=f"ohT{r}")
                nc.scalar.copy(ohT[:], psT[:])
                return ohT

            def origin_chain(po, ohT, r):
                """pooled origin -> svdd rows for round r."""
                b1 = chunk_layer(o1_sb, [po[:, g, :] for g in range(4)], 8, 128,
                                 r, "b1")
                b2 = chunk_layer(o2_sb, b1, 4, 128, r, "b2")
                ps = psA.tile([128, BC], f32, tag="mm")
                for m in range(4):
                    nc.tensor.matmul(ps[:64, :], o3_sb[:, m, :], b2[m],
                                     start=(m == 0), stop=(m == 3))
                orT = ap.tile([64, BC], bf16, tag=f"r{r}orT")
                nc.scalar.activation(orT[:], ps[:64, :], AF.Lrelu, alpha=0.01)

                # cfc (concat folded; -protos_c gathered via onehot matmul)
                cf1 = small_mm(c1_sb, orT, r, "cf1", extra=(pc_sb, ohT))
                clsT = small_mm(c2_sb, cf1, r, "cls")
                svdd_row(clsT, r, "c", off=(64 if r == 0 else 192))

                # oc head
                g1 = small_mm(q1_sb, orT, r, "g1")
                g2 = small_mm(q2_sb, g1, r, "g2")
                svdd_row(g2, r, "o", off=(0 if r == 0 else 128))

            # ---- emission: xi -> xo -> ai -> ao; the main-origin chain
            # overlaps the ai stream; only the aug chain is in the tail ----
            pi0 = pool_inter(xi, "pi0", after_first_pair=load_weights)
            m_sb, o1_sb = W["m"], W["o1"]
            s2_sb, o2_sb = W["s2"], W["o2"]
            s3_sb, o3_sb = W["s3"], W["o3"]
            sm_sb, sf_sb, pc_sb, p2_sb = W["sm"], W["sf"], W["pc"], W["p2"]
            t1_sb = sm_sb[:, 0:64]
            t2_sb = sm_sb[:, 64:128]
            c1_sb = sm_sb[:, 128:192]
            c2_sb = sm_sb[:, 192:256]
            q1_sb = sm_sb[:, 256:320]
            q2_sb = sm_sb[:, 320:384]
            id_sb = sf_sb[:, 0:64]
            bt1_sb = sf_sb[:, 64:65]
            bcn_sb = sf_sb[:, 65:66]
            it_sb = sf_sb[:, 66:67]

            po0 = pool_origin(xo, "po0")

            score0, mx0 = shallow_to_score(pi0, 0)
            ohT0 = onehot_T(score0, mx0, 0)
            origin_chain(po0, ohT0, 0)

            pi1 = pool_inter(ai, "pi1")
            score1, mx1 = shallow_to_score(pi1, 1)
            ohT1 = onehot_T(score1, mx1, 1)

            po1 = pool_origin(ao, "po1")
            origin_chain(po1, ohT1, 1)

            e0, es0, ses0 = stats_exp(score0, mx0, 0)
            e1, es1, ses1 = stats_exp(score1, mx1, 1)
            q0, ls0 = stats_fin(score0, mx0, e0, es0, ses0, 0)
            q1, ls1 = stats_fin(score1, mx1, e1, es1, ses1, 1)

            pr01 = sp.tile([64, 4], f32, tag="pr01")
            nc.vector.tensor_mul(pr01[:], q0[:], ls1[:])
            pc01 = psB.tile([1, 4], f32, tag="pc")
            nc.tensor.matmul(pc01[:], ones_sb[:], pr01[:])
            nc.vector.tensor_copy(out_sb[:, 256:260], pc01[:])

            pr10 = sp.tile([64, 4], f32, tag="pr10")
            nc.vector.tensor_mul(pr10[:], q1[:], ls0[:])
            pc10 = psB.tile([1, 4], f32, tag="pc")
            nc.tensor.matmul(pc10[:], ones_sb[:], pr10[:])
            nc.vector.tensor_copy(out_sb[:, 260:264], pc10[:])

            nc.sync.dma_start(out[:], out_sb[:])

    _split_waits(nc, mybir)
    return nc


def _get_nc():
    global _NC
    if _NC is None:
        _NC = _build_nc()
    return _NC


def _prep_weights(shallow_conv_w, ofc_w1, ofc_w2, ofc_w3, sfc_w1, sfc_w2, sfc_w3,
                  tfc_w1, tfc_w2, cfc_w1, cfc_w2, oc_w1, oc_w2, center, protos,
                  epoch):
    f = np.float32
    sw = np.asarray(shallow_conv_w, f)
    o1, o2, o3 = (np.asarray(a, f) for a in (ofc_w1, ofc_w2, ofc_w3))
    s1, s2, s3 = (np.asarray(a, f) for a in (sfc_w1, sfc_w2, sfc_w3))
    t1, t2 = np.asarray(tfc_w1, f), np.asarray(tfc_w2, f)
    c1, c2 = np.asarray(cfc_w1, f), np.asarray(cfc_w2, f)
    q1, q2 = np.asarray(oc_w1, f), np.asarray(oc_w2, f)
    ctr = np.asarray(center, f)
    pr = np.asarray(protos, f)

    import ml_dtypes
    bf = ml_dtypes.bfloat16

    w = {}
    # folded first stage: M = s1 @ (sw / HW2): [1024, 256]
    m = (s1 @ (sw / HW2)).astype(f)
    w["w_m"] = np.ascontiguousarray(
        m.T.reshape(2, 128, 1024).transpose(1, 0, 2).reshape(128, 2048)
    ).astype(bf)
    w["w_s2"] = np.ascontiguousarray(
        s2.T.reshape(8, 128, 512).transpose(1, 0, 2).reshape(128, 4096)
    ).astype(bf)
    w["w_s3"] = np.ascontiguousarray(
        s3.T.reshape(4, 128, 64).transpose(1, 0, 2).reshape(128, 256)
    ).astype(bf)
    w["w_o1"] = np.ascontiguousarray(
        (o1.T / HO2).astype(f).reshape(4, 128, 1024).transpose(1, 0, 2)
        .reshape(128, 4096)
    ).astype(bf)
    w["w_o2"] = np.ascontiguousarray(
        o2.T.reshape(8, 128, 512).transpose(1, 0, 2).reshape(128, 4096)
    ).astype(bf)
    w["w_o3"] = np.ascontiguousarray(
        o3.T.reshape(4, 128, 64).transpose(1, 0, 2).reshape(128, 256)
    ).astype(bf)
    ta, tb = t1[:, :64], t1[:, 64:]
    ca, cb = c1[:, :64], c1[:, 64:]
    # compacted small weights: [64, 384] bf16 = t1|t2|c1|c2|q1|q2
    w["w_sm"] = np.ascontiguousarray(np.concatenate(
        [(ta + tb).T, t2.T, (ca + cb).T, c2.T, q1.T, q2.T], axis=1)).astype(bf)
    temp = f(_SCHED[int(np.asarray(epoch))])
    # [64, 67] f32 = idm | b_t1 | b_cn | invt
    w["w_sf"] = np.ascontiguousarray(np.concatenate(
        [np.eye(64, dtype=f), -(tb @ ctr)[:, None], -ctr[:, None],
         np.full((64, 1), 1.0 / temp, f)], axis=1))
    w["w_pc"] = np.ascontiguousarray(-(pr @ cb.T)).astype(bf)
    p2 = np.concatenate([2.0 * pr.T, -(pr ** 2).sum(1)[None, :]], 0).astype(f)
    w["w_p2"] = np.ascontiguousarray(p2)
    return w


def _pack_inter(x):
    """[BC, CI*HW2] f32 -> [128, NT, TS*2*HW2] bf16; c = g*128 + p."""
    import ml_dtypes
    # [t, b, g, p, s] -> [p, t, b, g, s]
    a = x.reshape(NT, TS, 2, 128, HW2).transpose(3, 0, 1, 2, 4)
    return np.ascontiguousarray(a.astype(ml_dtypes.bfloat16)
                                .reshape(128, NT, TS * 2 * HW2))


def _pack_origin(x):
    """[BC, CO*HO2] f32 -> [128, 4*BC*HO2] bf16; c = g*128 + p."""
    import ml_dtypes
    # [b, g, p, s] -> [p, g, b, s]
    a = x.reshape(BC, 4, 128, HO2).transpose(2, 1, 0, 3)
    return np.ascontiguousarray(a.astype(ml_dtypes.bfloat16)
                                .reshape(128, 4 * BC * HO2))


def _run(inputs, trace=False):
    from concourse.bass_utils import run_bass_kernel_spmd

    nc = _get_nc()
    f = np.float32
    inter = np.asarray(inputs["inter_feat"], f).reshape(B, CI * HW2)
    orig = np.asarray(inputs["origin_feat"], f).reshape(B, CO * HO2)
    ainter = np.asarray(inputs["aug_inter_feat"], f).reshape(B, CI * HW2)
    aorig = np.asarray(inputs["aug_origin_feat"], f).reshape(B, CO * HO2)
    w = _prep_weights(
        inputs["shallow_conv_w"], inputs["ofc_w1"], inputs["ofc_w2"],
        inputs["ofc_w3"], inputs["sfc_w1"], inputs["sfc_w2"], inputs["sfc_w3"],
        inputs["tfc_w1"], inputs["tfc_w2"], inputs["cfc_w1"], inputs["cfc_w2"],
        inputs["oc_w1"], inputs["oc_w2"], inputs["center"], inputs["protos"],
        inputs["epoch"],
    )
    in_maps = []
    for c in range(NCORE):
        sl = slice(c * BC, (c + 1) * BC)
        m = dict(w)
        m["xi"] = _pack_inter(inter[sl])
        m["xo"] = _pack_origin(orig[sl])
        m["ai"] = _pack_inter(ainter[sl])
        m["ao"] = _pack_origin(aorig[sl])
        in_maps.append(m)

    res = run_bass_kernel_spmd(nc, in_maps, core_ids=list(range(NCORE)),
                               trace=trace)
    rows = np.stack([res.results[c]["out"][0] for c in range(NCORE)])  # [8, 264]
    osv0 = rows[:, 0:64].astype(f)
    csv0 = rows[:, 64:128].astype(f)
    osv1 = rows[:, 128:192].astype(f)
    csv1 = rows[:, 192:256].astype(f)
    s01 = rows[:, 256:260].astype(f)
    s10 = rows[:, 260:264].astype(f)

    l01 = f(-(s01.sum(dtype=f)) / B)
    l10 = f(-(s10.sum(dtype=f)) / B)
    distill = f((l01 + l10) / 2.0)
    row_o = f(osv0.sum(dtype=f) / B + osv1.sum(dtype=f) / B)
    row_c = f(csv0.sum(dtype=f) / B + csv1.sum(dtype=f) / B)
    row_a = f(np.abs(osv0 - csv0).sum(dtype=f) / B
              + np.abs(osv1 - csv1).sum(dtype=f) / B)
    out = np.array([[distill], [row_o], [row_c], [row_a]], dtype=f)
    return out, res


def kernel(**inputs):
    out, _ = _run(inputs, trace=False)
    return out


# revision 43
# speedup vs baseline: 1.1588x; 1.1588x over previous
"""Trainium2 Bass kernel for nn_DGAD_net (vq_codebook).

Strategy (v3)
-------------
The reference's dominant cost is ``einsum('bchw,oc->bohw', inter, w).mean((2,3))``
followed by tiny MLPs.  The einsum and the spatial mean commute (both linear),
so on device we only *sum-pool* the feature maps and fold ``w / (H*W)`` into the
first matmul.  v2 hit the f32 DMA roofline (120 MB/core at ~368 GB/s = 326 us),
so v3 halves the bytes:

- Features are host-cast to bf16 and host-packed so that every DMA is one
  contiguous >=12 KiB run per partition (descriptor-efficient; channel
  c = g*128 + p to match the weight packing).
- ``sfc_w1 @ (shallow_conv_w / HW)`` is folded on the host: kills the 0.25 MB
  conv weight + the whole 512-wide "sh" matmul stage.
- Pooling: InstTensorReduce has no DVE fast modes (1x = 123 G elem/s, would
  bottleneck at ~235 us), so the 16x49 spatial factor is reduced with a
  tensor_tensor add tree (2x_1p mode, bf16 packed operands) down to 1x49,
  then one 1x reduce.  ~6.7 Kcycles per 8-sample tile.

Data parallel over 8 NeuronCores: batch 512 -> 64 per core.  Each core
returns a [1, 264] row of per-sample partials:
  [osvdd_main(64) | csvdd_main(64) | osvdd_aug(64) | csvdd_aug(64)
   | sum(q0*ls1)(4) | sum(q1*ls0)(4)]
and the host reduces those to the [4, 1] output.

The distill term only needs softmax/log_softmax of ``sim`` which are invariant
to per-row shifts, so the device computes ``score[b,k] = 2 t.p_k - ||p_k||^2``
(skipping ``-||t||^2``) -- same softmax, same argmax.

Streams xi -> ai -> xo -> ao alternate over both HWDGE rings (weights up
front, split across rings); the long shallow/texture/distill chains overlap
the later streams so the kernel tail is only the short aug-origin chain.

This walrus build only encodes ONE sync wait per instruction; ``_split_waits``
rewrites the traced BIR, moving excess waits onto preceding same-engine NOPs.
"""

import sys

for _p in ("/opt/trn_rl_repo", "/root/.axon_site/_ro/trn_rl_repo"):
    if _p not in sys.path:
        sys.path.append(_p)

import numpy as np

B, CI, HW2, CO, HO2, D, DOM = 512, 256, 784, 512, 49, 64, 4
NCORE = 8
BC = B // NCORE  # 64 samples per core
TS = 8           # inter samples per tile
NT = BC // TS    # 8 inter tiles per stream
EPOCHS = 30
W_TEMP, T_TEMP = 0.7, 0.4
_SCHED = np.concatenate(
    [np.linspace(W_TEMP, T_TEMP, int(EPOCHS * 0.25)),
     np.ones(EPOCHS - int(EPOCHS * 0.25)) * T_TEMP]
)

_NC = None  # built once per process
_ctr = [0]


def _split_waits(nc, mybir, cap=1):
    """Move excess sync waits onto preceding same-engine NOPs (this
    walrus encodes at most `cap` waits per instruction).  Same-engine
    program order preserves semantics exactly."""
    for f in nc.m.functions:
        for bb in f.blocks:
            new = []
            for inst in bb.instructions:
                si = inst.sync_info
                if si is not None and si.on_wait and len(si.on_wait) > cap:
                    waits = list(si.on_wait)
                    excess, keep = waits[:-cap], waits[-cap:]
                    while excess:
                        chunk, excess = excess[:cap], excess[cap:]
                        nop = mybir.InstNoOp(
                            name=f"I-wsplit-{_ctr[0]}", ins=[], outs=[]
                        )
                        _ctr[0] += 1
                        nop.engine = inst.engine
                        nop.sync_info = mybir.SyncInfo(on_wait=chunk, on_update=[])
                        new.append(nop)
                    inst.sync_info = mybir.SyncInfo(
                        on_wait=keep, on_update=list(si.on_update)
                    )
                new.append(inst)
            bb.instructions = new


def _build_nc():
    import concourse.bass as bass
    import concourse.tile as tile
    from concourse import mybir
    from contextlib import ExitStack

    AF = mybir.ActivationFunctionType
    AL = mybir.AluOpType
    AX = mybir.AxisListType
    f32 = mybir.dt.float32
    bf16 = mybir.dt.bfloat16

    nc = bass.Bass(trn_type="TRN2")

    # ---- DRAM I/O ----
    # inter: [p, tile, b(8), g(2), s(784)] bf16, channel c = g*128 + p
    xi = nc.dram_tensor("xi", [128, NT, TS * 2 * HW2], bf16, kind="ExternalInput")
    ai = nc.dram_tensor("ai", [128, NT, TS * 2 * HW2], bf16, kind="ExternalInput")
    # origin: [p, g(4), b(64), s(49)] bf16, channel c = g*128 + p
    xo = nc.dram_tensor("xo", [128, 4 * BC * HO2], bf16, kind="ExternalInput")
    ao = nc.dram_tensor("ao", [128, 4 * BC * HO2], bf16, kind="ExternalInput")
    # weights: host-packed flat [128, X] (per-partition contiguous)
    w_m = nc.dram_tensor("w_m", [128, 2 * 1024], bf16, kind="ExternalInput")
    w_s2 = nc.dram_tensor("w_s2", [128, 8 * 512], bf16, kind="ExternalInput")
    w_s3 = nc.dram_tensor("w_s3", [128, 4 * 64], bf16, kind="ExternalInput")
    w_o1 = nc.dram_tensor("w_o1", [128, 4 * 1024], bf16, kind="ExternalInput")
    w_o2 = nc.dram_tensor("w_o2", [128, 8 * 512], bf16, kind="ExternalInput")
    w_o3 = nc.dram_tensor("w_o3", [128, 4 * 64], bf16, kind="ExternalInput")
    # small weights compacted: [64, 384] bf16 = t1|t2|c1|c2|q1|q2,
    # [64, 67] f32 = idm | b_t1 | b_cn | invt
    w_sm = nc.dram_tensor("w_sm", [64, 384], bf16, kind="ExternalInput")
    w_sf = nc.dram_tensor("w_sf", [64, 67], f32, kind="ExternalInput")
    w_pc = nc.dram_tensor("w_pc", [4, 64], bf16, kind="ExternalInput")
    w_p2 = nc.dram_tensor("w_p2", [65, 4], f32, kind="ExternalInput")
    out = nc.dram_tensor("out", [1, 264], f32, kind="ExternalOutput")

    from concourse.tile import add_dep_helper

    with tile.TileContext(nc) as tc:
        with ExitStack() as ctx:
            wp = ctx.enter_context(tc.tile_pool(name="wp", bufs=1))
            iop = ctx.enter_context(tc.tile_pool(name="iop", bufs=5))
            orp = ctx.enter_context(tc.tile_pool(name="orp", bufs=2))
            plp = ctx.enter_context(tc.tile_pool(name="plp", bufs=1))
            ap = ctx.enter_context(tc.tile_pool(name="ap", bufs=1))
            sp = ctx.enter_context(tc.tile_pool(name="sp", bufs=1))
            psA = ctx.enter_context(tc.tile_pool(name="psA", bufs=4, space="PSUM"))
            psB = ctx.enter_context(tc.tile_pool(name="psB", bufs=1, space="PSUM"))

            # ---- weights: on the ACT (scalar) HWDGE ring, which is free of
            # other DMA traffic; they stream concurrently with the first
            # feature tiles on the sync ring and are done long before any
            # ACT compute shows up on that sequencer ----
            def wtile(h, shape, tag, dt=f32):
                t = wp.tile(shape, dt, tag=tag)
                dst = t[:]
                if len(shape) == 3:
                    dst = dst.rearrange("p m j -> p (m j)")
                nc.scalar.dma_start(dst, h[:])
                return t

            W = {}

            def load_weights():
                W["m"] = wtile(w_m, [128, 2, 1024], "w_m", bf16)
                W["o1"] = wtile(w_o1, [128, 4, 1024], "w_o1", bf16)
                W["s2"] = wtile(w_s2, [128, 8, 512], "w_s2", bf16)
                W["o2"] = wtile(w_o2, [128, 8, 512], "w_o2", bf16)
                W["s3"] = wtile(w_s3, [128, 4, 64], "w_s3", bf16)
                W["o3"] = wtile(w_o3, [128, 4, 64], "w_o3", bf16)
                W["sm"] = wtile(w_sm, [64, 384], "w_sm", bf16)
                W["sf"] = wtile(w_sf, [64, 67], "w_sf")
                W["pc"] = wtile(w_pc, [4, 64], "w_pc", bf16)
                W["p2"] = wtile(w_p2, [65, 4], "w_p2")

            ones_sb = wp.tile([64, 1], f32, tag="ones")
            nc.vector.memset(ones_sb[:], 1.0)
            out_sb = wp.tile([1, 264], f32, tag="out_sb")

            # ---- pooling.  The early stream portion (xi/xo, done before any
            # ACT compute exists) alternates over both HWDGE rings to hide
            # per-DMA completion tails; the late portion (ai/ao) stays on the
            # sync ring, whose sequencer runs no compute.  The add tree runs
            # in place inside the io tile so each tile's tree is fully
            # independent. ----
            def pool_inter(xh, tag, ring2=False, after_first_pair=None):
                """[128, 2, BC] bf16 pooled sums; channel c = g*128 + p."""
                dst = plp.tile([128, 2, BC], bf16, tag=tag)
                for t in range(NT):
                    tl = iop.tile([128, 2 * TS, 16, HO2], bf16, tag="io")
                    eng = nc.scalar if (ring2 and t % 2 == 1) else nc.sync
                    eng.dma_start(
                        tl[:].rearrange("p m k s -> p (m k s)"), xh[:, t, :]
                    )
                    if t == 1 and after_first_pair is not None:
                        after_first_pair()
                    # in-place add tree over the 16 spatial chunks (2x_1p bf16)
                    nc.vector.tensor_add(
                        tl[:, :, 0:8, :], tl[:, :, 0:8, :], tl[:, :, 8:16, :]
                    )
                    nc.vector.tensor_add(
                        tl[:, :, 0:4, :], tl[:, :, 0:4, :], tl[:, :, 4:8, :]
                    )
                    nc.vector.tensor_add(
                        tl[:, :, 0:2, :], tl[:, :, 0:2, :], tl[:, :, 2:4, :]
                    )
                    nc.vector.tensor_add(
                        tl[:, :, 0:1, :], tl[:, :, 0:1, :], tl[:, :, 1:2, :]
                    )
                    with nc.allow_low_precision(reason="pooled sums in bf16"):
                        nc.vector.reduce_sum(
                            dst[:, :, t * TS:(t + 1) * TS]
                            .rearrange("p g b -> p b g"),
                            tl[:, :, 0:1, :],
                            axis=AX.X,
                        )
                return dst

            def pool_origin(xh, tag, ring2=False):
                """[128, 4, BC] bf16 pooled sums; channel c = g*128 + p."""
                dst = plp.tile([128, 4, BC], bf16, tag=tag)
                for g in range(4):  # one channel-group per chunk
                    tl = orp.tile([128, BC, HO2], bf16, tag="or")
                    eng = nc.scalar if (ring2 and g % 2 == 1) else nc.sync
                    eng.dma_start(
                        tl[:].rearrange("p b s -> p (b s)"),
                        xh[:, g * BC * HO2:(g + 1) * BC * HO2],
                    )
                    nc.vector.tensor_add(
                        tl[:, :, 0:24], tl[:, :, 0:24], tl[:, :, 24:48]
                    )
                    nc.vector.tensor_add(
                        tl[:, :, 0:12], tl[:, :, 0:12], tl[:, :, 12:24]
                    )
                    with nc.allow_low_precision(reason="pooled sums in bf16"):
                        nc.vector.reduce_sum(
                            dst[:, g, :], tl[:, :, 0:12], axis=AX.X,
                        )
                    nc.vector.tensor_add(
                        dst[:, g, :], dst[:, g, :], tl[:, :, 48]
                    )
                return dst

            # ---- MLP helpers ----
            def chunk_layer(w_sb, ins, nout_chunks, outw, r, nm, act=True):
                """outT chunks [outw, BC] = Lrelu( sum_m w_sb[:, m, chunk] @ ins[m] )."""
                outs = []
                nin = len(ins)
                for m2 in range(nout_chunks):
                    ps = psA.tile([128, BC], f32, tag="mm")
                    for m in range(nin):
                        nc.tensor.matmul(
                            ps[:outw, :],
                            w_sb[:, m, m2 * outw:(m2 + 1) * outw],
                            ins[m],
                            start=(m == 0),
                            stop=(m == nin - 1),
                        )
                    tl = ap.tile([outw, BC], bf16, tag=f"r{r}{nm}{m2}")
                    if act:
                        nc.scalar.activation(tl[:], ps[:outw, :], AF.Lrelu,
                                             alpha=0.01)
                    else:
                        nc.scalar.copy(tl[:], ps[:outw, :])
                    outs.append(tl[:])
                return outs

            def small_mm(lhsT, rhs, r, nm, act=True, bias=None, extra=None):
                """[64, BC] = act(lhsT.T @ rhs [+ extra matmul] + bias)."""
                ps = psA.tile([128, BC], f32, tag="mm")
                nc.tensor.matmul(
                    ps[:64, :], lhsT[:], rhs[:],
                    start=True, stop=(extra is None),
                )
                if extra is not None:
                    nc.tensor.matmul(
                        ps[:64, :], extra[0][:], extra[1][:], start=False, stop=True
                    )
                tl = ap.tile([64, BC], bf16, tag=f"r{r}{nm}")
                if bias is not None:
                    nc.scalar.activation(
                        tl[:], ps[:64, :], AF.Lrelu, bias=bias[:], alpha=0.01
                    )
                else:
                    nc.scalar.activation(tl[:], ps[:64, :], AF.Lrelu, alpha=0.01)
                return tl

            def svdd_row(featT, r, nm, off):
                """out_sb[0, off:off+64] = || featT[:, b] - center ||^2 per b.

                Round 0 runs mid-stream: its ops go to gpsimd (slow but off
                the vector queue, which must stay clear for the add trees).
                Round 1 runs in the tail where vector is idle and fast."""
                ve = nc.gpsimd if r == 0 else nc.vector
                df = sp.tile([64, BC], f32, tag=f"r{r}{nm}df")
                ve.tensor_scalar(df[:], featT[:], bcn_sb[:], None, op0=AL.add)
                sq = sp.tile([64, BC], f32, tag=f"r{r}{nm}sq")
                ve.tensor_mul(sq[:], df[:], df[:])
                pr = psB.tile([1, BC], f32, tag="row")
                nc.tensor.matmul(pr[:], ones_sb[:], sq[:])
                # PSUM source: gpsimd cannot read PSUM -> ACT for round 0
                if r == 0:
                    nc.scalar.copy(out_sb[:, off:off + BC], pr[:])
                else:
                    nc.vector.tensor_copy(out_sb[:, off:off + BC], pr[:])

            def stats_exp(score, mx, r):
                """Exp phase for teacher q and student log-softmax (both
                rounds' Exps are emitted together: one ACT table load)."""
                nb = sp.tile([64, 1], f32, tag=f"nb{r}")
                nc.vector.tensor_scalar(nb[:], mx[:], it_sb[:], -1.0,
                                        op0=AL.mult, op1=AL.mult)
                e = sp.tile([64, 4], f32, tag=f"te{r}")
                es = sp.tile([64, 1], f32, tag=f"tes{r}")
                nc.scalar.activation(e[:], score[:], AF.Exp, bias=nb[:],
                                     scale=it_sb[:], accum_out=es[:])
                snb = sp.tile([64, 1], f32, tag=f"snb{r}")
                nc.vector.tensor_scalar(snb[:], mx[:], -1.0, None, op0=AL.mult)
                se = sp.tile([64, 4], f32, tag=f"se{r}")
                ses = sp.tile([64, 1], f32, tag=f"ses{r}")
                nc.scalar.activation(se[:], score[:], AF.Exp, bias=snb[:],
                                     accum_out=ses[:])
                return e, es, ses

            def stats_fin(score, mx, e, es, ses, r):
                """Ln phase + q/ls products (vector)."""
                rc = sp.tile([64, 1], f32, tag=f"trc{r}")
                nc.vector.reciprocal(rc[:], es[:])
                q = sp.tile([64, 4], f32, tag=f"tq{r}")
                nc.vector.tensor_scalar(q[:], e[:], rc[:], None, op0=AL.mult)
                ln = sp.tile([64, 1], f32, tag=f"sln{r}")
                nc.scalar.activation(ln[:], ses[:], AF.Ln)
                lse = sp.tile([64, 1], f32, tag=f"slse{r}")
                nc.vector.tensor_add(lse[:], ln[:], mx[:])
                ls = sp.tile([64, 4], f32, tag=f"sls{r}")
                nc.vector.tensor_scalar(ls[:], score[:], lse[:], None,
                                        op0=AL.subtract)
                return q, ls

            def shallow_to_score(pi, r):
                """pooled inter -> score/mx for round r (stats done later)."""
                a1 = chunk_layer(m_sb, [pi[:, g, :] for g in range(2)], 8, 128,
                                 r, "a1")
                a2 = chunk_layer(s2_sb, a1, 4, 128, r, "a2")
                ps = psA.tile([128, BC], f32, tag="mm")
                for m in range(4):
                    nc.tensor.matmul(ps[:64, :], s3_sb[:, m, :], a2[m],
                                     start=(m == 0), stop=(m == 3))
                sT = ap.tile([64, BC], bf16, tag=f"r{r}sT")
                nc.scalar.activation(sT[:], ps[:64, :], AF.Lrelu, alpha=0.01)

                # texture MLP (concat folded into w_t1 + bias)
                t1 = small_mm(t1_sb, sT, r, "t1", bias=bt1_sb)
                ps = psA.tile([128, BC], f32, tag="mm")
                nc.tensor.matmul(ps[:64, :], t2_sb[:], t1[:])
                tx = ap.tile([65, BC], f32, tag=f"r{r}tx")
                nc.scalar.activation(tx[0:64, :], ps[:64, :], AF.Lrelu, alpha=0.01)
                nc.vector.memset(tx[64:65, :], 1.0)

                # score[b, k] = 2 t.p_k - ||p_k||^2.  Everything below runs
                # on gpsimd: these ops wait on the PE chain, and on the
                # in-order vector queue they would block later tree work.
                pss = psB.tile([64, 4], f32, tag="sc")
                nc.tensor.matmul(pss[:], tx[:], p2_sb[:])
                score = sp.tile([64, 4], f32, tag=f"score{r}")
                nc.scalar.copy(score[:], pss[:])
                mx = sp.tile([64, 1], f32, tag=f"mx{r}")
                nc.vector.reduce_max(mx[:], score[:], axis=AX.X)
                return score, mx

            def onehot_T(score, mx, r):
                """argmax one-hot, transposed -- gpsimd + PE only."""
                oh1 = sp.tile([64, 4], f32, tag=f"oh{r}")
                nc.gpsimd.tensor_scalar(oh1[:], score[:], mx[:], None,
                                        op0=AL.is_ge)
                psT = psB.tile([4, 64], f32, tag="ohT")
                nc.tensor.transpose(psT[:], oh1[:], id_sb[:])
                ohT = sp.tile([4, 64], bf16, tag=f"ohT{r}")
                nc.scalar.copy(ohT[:], psT[:])
                return ohT

            def origin_chain(po, ohT, r):
                """pooled origin -> svdd rows for round r."""
                b1 = chunk_layer(o1_sb, [po[:, g, :] for g in range(4)], 8, 128,
                                 r, "b1")
                b2 = chunk_layer(o2_sb, b1, 4, 128, r, "b2")
                ps = psA.tile([128, BC], f32, tag="mm")
                for m in range(4):
                    nc.tensor.matmul(ps[:64, :], o3_sb[:, m, :], b2[m],
                                     start=(m == 0), stop=(m == 3))
                orT = ap.tile([64, BC], bf16, tag=f"r{r}orT")
                nc.scalar.activation(orT[:], ps[:64, :], AF.Lrelu, alpha=0.01)

                # cfc (concat folded; -protos_c gathered via onehot matmul)
                cf1 = small_mm(c1_sb, orT, r, "cf1", extra=(pc_sb, ohT))
                clsT = small_mm(c2_sb, cf1, r, "cls")
                svdd_row(clsT, r, "c", off=(64 if r == 0 else 192))

                # oc head
                g1 = small_mm(q1_sb, orT, r, "g1")
                g2 = small_mm(q2_sb, g1, r, "g2")
                svdd_row(g2, r, "o", off=(0 if r == 0 else 128))

            # ---- emission: xi -> xo -> ai -> ao; the main-origin chain
            # overlaps the ai stream; only the aug chain is in the tail ----
            pi0 = pool_inter(xi, "pi0", after_first_pair=load_weights)
            m_sb, o1_sb = W["m"], W["o1"]
            s2_sb, o2_sb = W["s2"], W["o2"]
            s3_sb, o3_sb = W["s3"], W["o3"]
            sm_sb, sf_sb, pc_sb, p2_sb = W["sm"], W["sf"], W["pc"], W["p2"]
            t1_sb = sm_sb[:, 0:64]
            t2_sb = sm_sb[:, 64:128]
            c1_sb = sm_sb[:, 128:192]
            c2_sb = sm_sb[:, 192:256]
            q1_sb = sm_sb[:, 256:320]
            q2_sb = sm_sb[:, 320:384]
            id_sb = sf_sb[:, 0:64]
            bt1_sb = sf_sb[:, 64:65]
            bcn_sb = sf_sb[:, 65:66]
            it_sb = sf_sb[:, 66:67]

            po0 = pool_origin(xo, "po0")

            score0, mx0 = shallow_to_score(pi0, 0)
            ohT0 = onehot_T(score0, mx0, 0)
            origin_chain(po0, ohT0, 0)

            pi1 = pool_inter(ai, "pi1")
            score1, mx1 = shallow_to_score(pi1, 1)
            ohT1 = onehot_T(score1, mx1, 1)

            po1 = pool_origin(ao, "po1")
            origin_chain(po1, ohT1, 1)

            e0, es0, ses0 = stats_exp(score0, mx0, 0)
            e1, es1, ses1 = stats_exp(score1, mx1, 1)
            q0, ls0 = stats_fin(score0, mx0, e0, es0, ses0, 0)
            q1, ls1 = stats_fin(score1, mx1, e1, es1, ses1, 1)

            pr01 = sp.tile([64, 4], f32, tag="pr01")
            nc.vector.tensor_mul(pr01[:], q0[:], ls1[:])
            pc01 = psB.tile([1, 4], f32, tag="pc")
            nc.tensor.matmul(pc01[:], ones_sb[:], pr01[:])
            nc.vector.tensor_copy(out_sb[:, 256:260], pc01[:])

            pr10 = sp.tile([64, 4], f32, tag="pr10")
            nc.vector.tensor_mul(pr10[:], q1[:], ls0[:])
            pc10 = psB.tile([1, 4], f32, tag="pc")
            nc.tensor.matmul(pc10[:], ones_sb[:], pr10[:])
            nc.vector.tensor_copy(out_sb[:, 260:264], pc10[:])

            nc.sync.dma_start(out[:], out_sb[:])

    _split_waits(nc, mybir)
    return nc


def _get_nc():
    global _NC
    if _NC is None:
        _NC = _build_nc()
    return _NC


def _prep_weights(shallow_conv_w, ofc_w1, ofc_w2, ofc_w3, sfc_w1, sfc_w2, sfc_w3,
                  tfc_w1, tfc_w2, cfc_w1, cfc_w2, oc_w1, oc_w2, center, protos,
                  epoch):
    f = np.float32
    sw = np.asarray(shallow_conv_w, f)
    o1, o2, o3 = (np.asarray(a, f) for a in (ofc_w1, ofc_w2, ofc_w3))
    s1, s2, s3 = (np.asarray(a, f) for a in (sfc_w1, sfc_w2, sfc_w3))
    t1, t2 = np.asarray(tfc_w1, f), np.asarray(tfc_w2, f)
    c1, c2 = np.asarray(cfc_w1, f), np.asarray(cfc_w2, f)
    q1, q2 = np.asarray(oc_w1, f), np.asarray(oc_w2, f)
    ctr = np.asarray(center, f)
    pr = np.asarray(protos, f)

    import ml_dtypes
    bf = ml_dtypes.bfloat16

    w = {}
    # folded first stage: M = s1 @ (sw / HW2): [1024, 256]
    m = (s1 @ (sw / HW2)).astype(f)
    w["w_m"] = np.ascontiguousarray(
        m.T.reshape(2, 128, 1024).transpose(1, 0, 2).reshape(128, 2048)
    ).astype(bf)
    w["w_s2"] = np.ascontiguousarray(
        s2.T.reshape(8, 128, 512).transpose(1, 0, 2).reshape(128, 4096)
    ).astype(bf)
    w["w_s3"] = np.ascontiguousarray(
        s3.T.reshape(4, 128, 64).transpose(1, 0, 2).reshape(128, 256)
    ).astype(bf)
    w["w_o1"] = np.ascontiguousarray(
        (o1.T / HO2).astype(f).reshape(4, 128, 1024).transpose(1, 0, 2)
        .reshape(128, 4096)
    ).astype(bf)
    w["w_o2"] = np.ascontiguousarray(
        o2.T.reshape(8, 128, 512).transpose(1, 0, 2).reshape(128, 4096)
    ).astype(bf)
    w["w_o3"] = np.ascontiguousarray(
        o3.T.reshape(4, 128, 64).transpose(1, 0, 2).reshape(128, 256)
    ).astype(bf)
    ta, tb = t1[:, :64], t1[:, 64:]
    ca, cb = c1[:, :64], c1[:, 64:]
    # compacted small weights: [64, 384] bf16 = t1|t2|c1|c2|q1|q2
    w["w_sm"] = np.ascontiguousarray(np.concatenate(
        [(ta + tb).T, t2.T, (ca + cb).T, c2.T, q1.T, q2.T], axis=1)).astype(bf)
    temp = f(_SCHED[int(np.asarray(epoch))])
    # [64, 67] f32 = idm | b_t1 | b_cn | invt
    w["w_sf"] = np.ascontiguousarray(np.concatenate(
        [np.eye(64, dtype=f), -(tb @ ctr)[:, None], -ctr[:, None],
         np.full((64, 1), 1.0 / temp, f)], axis=1))
    w["w_pc"] = np.ascontiguousarray(-(pr @ cb.T)).astype(bf)
    p2 = np.concatenate([2.0 * pr.T, -(pr ** 2).sum(1)[None, :]], 0).astype(f)
    w["w_p2"] = np.ascontiguousarray(p2)
    return w


def _pack_inter(x):
    """[BC, CI*HW2] f32 -> [128, NT, TS*2*HW2] bf16; c = g*128 + p."""
    import ml_dtypes
    # [t, b, g, p, s] -> [p, t, b, g, s]
    a = x.reshape(NT, TS, 2, 128, HW2).transpose(3, 0, 1, 2, 4)
    return np.ascontiguousarray(a.astype(ml_dtypes.bfloat16)
                                .reshape(128, NT, TS * 2 * HW2))


def _pack_origin(x):
    """[BC, CO*HO2] f32 -> [128, 4*BC*HO2] bf16; c = g*128 + p."""
    import ml_dtypes
    # [b, g, p, s] -> [p, g, b, s]
    a = x.reshape(BC, 4, 128, HO2).transpose(2, 1, 0, 3)
    return np.ascontiguousarray(a.astype(ml_dtypes.bfloat16)
                                .reshape(128, 4 * BC * HO2))


def _run(inputs, trace=False):
    from concourse.bass_utils import run_bass_kernel_spmd

    nc = _get_nc()
    f = np.float32
    inter = np.asarray(inputs["inter_feat"], f).reshape(B, CI * HW2)
    orig = np.asarray(inputs["origin_feat"], f).reshape(B, CO * HO2)
    ainter = np.asarray(inputs["aug_inter_feat"], f).reshape(B, CI * HW2)
    aorig = np.asarray(inputs["aug_origin_feat"], f).reshape(B, CO * HO2)
    w = _prep_weights(
        inputs["shallow_conv_w"], inputs["ofc_w1"], inputs["ofc_w2"],
        inputs["ofc_w3"], inputs["sfc_w1"], inputs["sfc_w2"], inputs["sfc_w3"],
        inputs["tfc_w1"], inputs["tfc_w2"], inputs["cfc_w1"], inputs["cfc_w2"],
        inputs["oc_w1"], inputs["oc_w2"], inputs["center"], inputs["protos"],
        inputs["epoch"],
    )
    in_maps = []
    for c in range(NCORE):
        sl = slice(c * BC, (c + 1) * BC)
        m = dict(w)
        m["xi"] = _pack_inter(inter[sl])
        m["xo"] = _pack_origin(orig[sl])
        m["ai"] = _pack_inter(ainter[sl])
        m["ao"] = _pack_origin(aorig[sl])
        in_maps.append(m)

    res = run_bass_kernel_spmd(nc, in_maps, core_ids=list(range(NCORE)),
                               trace=trace)
    rows = np.stack([res.results[c]["out"][0] for c in range(NCORE)])  # [8, 264]
    osv0 = rows[:, 0:64].astype(f)
    csv0 = rows[:, 64:128].astype(f)
    osv1 = rows[:, 128:192].astype(f)
    csv1 = rows[:, 192:256].astype(f)
    s01 = rows[:, 256:260].astype(f)
    s10 = rows[:, 260:264].astype(f)

    l01 = f(-(s01.sum(dtype=f)) / B)
    l10 = f(-(s10.sum(dtype=f)) / B)
    distill = f((l01 + l10) / 2.0)
    row_o = f(osv0.sum(dtype=f) / B + osv1.sum(dtype=f) / B)
    row_c = f(csv0.sum(dtype=f) / B + csv1.sum(dtype=f) / B)
    row_a = f(np.abs(osv0 - csv0).sum(dtype=f) / B
              + np.abs(osv1 - csv1).sum(dtype=f) / B)
    out = np.array([[distill], [row_o], [row_c], [row_a]], dtype=f)
    return out, res


def kernel(**inputs):
    out, _ = _run(inputs, trace=False)
    return out
